# revision 31
# baseline (speedup 1.0000x reference)
"""Trainium2 Bass kernel for the pairwise contact-map decoder.

Reference computation (per batch b):
    tmp[b,i,c,h] = sum_a z[b,i,a] * W1[(a,c),h]
    h1[b,i,j,h]  = relu(sum_c tmp[b,i,c,h] * z[b,j,c] + b1[h])
    h2[b,i,j,k]  = relu(sum_h h1[b,i,j,h] * W2[h,k] + b2[k])
    logit[b,i,j] = (sum_k h2[b,i,j,k] * W3[k,0] + b3) * motif[b,i] * motif[b,j]
    cmap         = sigmoid(logit)

Key structural fact: the outer motif mask zeroes every (i,j) where either
index is masked out, making logit exactly 0 and cmap exactly sigmoid(0)=0.5
there.  So only the active submatrix (rows/cols with motif==1, ~50% each,
~25% of the grid) ever needs computing.  The host gathers active rows/cols,
the device computes the dense active block, and the host scatters results
into a 0 / 0.5 - filled full-size output.  Exact: masked entries match the
reference bit-for-bit, active entries follow the same fp16 path as before.

Sharding: 8 cores, core = 2*b + half; the two cores of a batch split its
active rows.  Compiled slab: ROWS=72 i-rows x NJ=144 j-cols (covers any
mask with <=144 active per batch; a full-size 128x256 variant is built
lazily as fallback for larger masks).

On-core dataflow (identical pipeline to the full-grid version):
  stage A (float32r matmuls): tmp2[i, c, h] staged to an fp16 DRAM scratch
           with an extra c-row holding b1 (bias folded via K=33).
  per i-pair: stage B  h1T[h,(i,j)] = tmp2_i.T @ zTx  (K=33 includes bias)
              stage C  h2T[k,(i,j)] accumulate over 4 h-chunks of W2
              stage D  logits strip (1, 2*NJ) via W3 chunks
  Stage C/D of pair p runs after stage B of pair p+1 (software pipeline).
  epilogue: sigmoid + DMA out in row-halves (no mask work on device).
"""

import numpy as np

import concourse.bass as bass
import concourse.mybir as mybir
import concourse.tile as tile
from concourse import bacc
from concourse.bass_utils import run_bass_kernel_spmd

B, N, D, H = 4, 256, 32, 512
DT = mybir.dt
F32, F32R, F16 = DT.float32, DT.float32r, DT.float16
AF = mybir.ActivationFunctionType
ALU = mybir.AluOpType
NCORES = 8
ROWS = 72   # padded active i-rows per core
NJ = 144    # padded active j-cols per batch

_cached_nc = {}


from contextlib import nullcontext as _nullcontext


def _r(ap):
    return ap.bitcast(F32R)


def _build(reps=1, rows=ROWS, nj=NJ, unroll=False):
    npair = rows // 2
    nc = bacc.Bacc("TRN2", target_bir_lowering=False, debug=False, num_devices=NCORES)

    ziT = nc.dram_tensor("ziT", [D, rows], F16, kind="ExternalInput")
    zTx = nc.dram_tensor("zTx", [D + 1, nj], F32, kind="ExternalInput")
    W1 = nc.dram_tensor("W1", [D * D, H], F16, kind="ExternalInput")
    W2 = nc.dram_tensor("W2", [H, H // 2], F32, kind="ExternalInput")
    W3 = nc.dram_tensor("W3", [H // 2, 1], F32, kind="ExternalInput")
    b1 = nc.dram_tensor("b1", [H], F32, kind="ExternalInput")
    b2 = nc.dram_tensor("b2", [H // 2], F32, kind="ExternalInput")
    b3 = nc.dram_tensor("b3", [1], F32, kind="ExternalInput")
    logits_o = nc.dram_tensor("logits", [rows, nj], F32, kind="ExternalOutput")
    cmap_o = nc.dram_tensor("cmap", [rows, nj], F32, kind="ExternalOutput")
    # c-major scratch: transpose lives on the DRAM-side write AP, reads are
    # clean strided loads (SBUF APs cannot cross partitions)
    tmp2xT = nc.dram_tensor("tmp2xT", [D, rows, H], F16)

    with tile.TileContext(nc) as tc:
        with (
            tc.tile_pool(name="const", bufs=1) as cp,
            tc.tile_pool(name="work", bufs=3) as wp,
            tc.tile_pool(name="ps", bufs=2, space="PSUM") as ps,
        ):
          rep_ctx = (
              tc.For_i(0, reps, 1) if reps > 1 and not unroll else _nullcontext()
          )
          for _rep in range(reps if unroll else 1):
           with rep_ctx if _rep == 0 else _nullcontext():
              # ---------- persistent loads ----------
              ziT_s = cp.tile([D, rows], F16)
              nc.sync.dma_start(ziT_s[:], ziT.ap())
              W1v = W1.ap().rearrange("(a c) h -> a c h", a=D)
              W1_s = cp.tile([D, D, H], F16)
              nc.sync.dma_start(W1_s[:, 0:4, :], W1v[:, 0:4, :])
              nc.sync.dma_start(W1_s[:, 4:8, :], W1v[:, 4:8, :])
              for q in range(1, 4):
                  nc.sync.dma_start(W1_s[:, 8 * q : 8 * (q + 1), :], W1v[:, 8 * q : 8 * (q + 1), :])
              zTx_s = cp.tile([D + 1, nj], F16)
              nc.gpsimd.dma_start(zTx_s[:], zTx.ap())
              W2_s = cp.tile([128, 4, 256], F16)
              nc.gpsimd.dma_start(W2_s[:], W2.ap().rearrange("(c p) k -> p c k", c=4))
              W3_s = cp.tile([128, 2], F16)
              nc.gpsimd.dma_start(W3_s[:], W3.ap().rearrange("(c p) o -> p (c o)", c=2))
              b2_s = cp.tile([128, 2], F32)
              nc.sync.dma_start(b2_s[:], b2.ap().rearrange("(c p) -> p c", c=2))
              b3_s = cp.tile([1, 1], F32)
              nc.sync.dma_start(b3_s[:], b3.ap().unsqueeze(0))
              # pair-major logits: partition p holds rows 2p,2p+1 so the
              # stage-D strip activation writes it directly (no SBUF->SBUF
              # DMA); the output DMA un-reshapes via the DRAM-side AP
              logits_sb = cp.tile([npair, 2 * nj], F32)

              # tpfull: 3-deep manual stationary ring whose last partition
              # row permanently holds b1 (the K=33 bias trick lives in the
              # stationary, so it is filled only once, not per pair)
              TPB = 3
              tpfull = cp.tile([D + 1, TPB, 2, 2, H], F16)
              for t in range(TPB):
                  nc.gpsimd.dma_start(
                      tpfull[D : D + 1, t, :, :, :],
                      b1.ap().unsqueeze(0).broadcast_to([4, H]).unsqueeze(0),
                  )

              # ---------- stage A: tmp2xT[c, i, :] ----------
              sbA = None
              for n in range(D):
                  psA = ps.tile([rows, H], F32, tag="ac", bufs=3)
                  nc.tensor.matmul(psA[:], ziT_s[:], W1_s[:, n, :], start=True, stop=True)
                  if n % 2 == 0:
                      sbA = wp.tile([rows, 2, H], F16, tag="sa", bufs=6)
                      nc.vector.tensor_copy(sbA[:, 0, :], psA[:])
                  else:
                      nc.scalar.copy(sbA[:, 1, :], psA[:])
                      # SP/HWDGE; transpose to c-major on the DRAM-side AP
                      nc.sync.dma_start(
                          tmp2xT.ap()[n - 1 : n + 1, :, :].rearrange("c i h -> i c h"),
                          sbA[:],
                      )

              # ---------- main loop over i-pairs (software-pipelined) ----------
              # 3-stage pipeline: B(p), C(p-1), D(p-2).  Each cross-engine
              # handoff (PE->DVE/Act evict -> PE) gets a full pair of slack;
              # HW semaphore latency is far larger than the cost model's
              # 100ns, so depth, not speed, is what hides it.
              def tp_fetch(g):
                  # one gather covers two pairs (group g)
                  if g * 2 < npair:
                      nc.sync.dma_start(
                          tpfull[0:D, g % TPB, :, :, :],
                          tmp2xT.ap()[:, 4 * g : 4 * g + 4, :].rearrange(
                              "c (p i) h -> c p i h", p=2
                          ),
                      )

              def stage_B(p):
                  if p % 2 == 0:
                      tp_fetch(p // 2 + 2)  # prefetch two groups (4 pairs) ahead
                  tp = tpfull[:, (p // 2) % TPB, p % 2]
                  h1T = wp.tile([128, 4, 2 * nj], F16, tag="h1", bufs=5)
                  for i in range(2):
                      # quarters padded to 256 so each matmul output stays
                      # inside one 2KB PSUM bank (accumulation is per-bank)
                      psB = ps.tile([128, 4, 256], F32, tag="b", bufs=2)
                      for hc in range(4):
                          nc.tensor.matmul(
                              psB[:, hc, 0:nj],
                              tp[:, i, hc * 128 : (hc + 1) * 128],
                              zTx_s[:],
                              start=(hc % 2 == 0),
                              stop=(hc % 2 == 1),
                          )
                      # relu; bias already folded in via the K=33 ones row
                      nc.vector.tensor_scalar(
                          h1T[:, :, i * nj : (i + 1) * nj], psB[:, :, 0:nj],
                          0.0, None, ALU.max,
                      )
                  return h1T

              def stage_C(p, h1T):
                  h2T = wp.tile([128, 2, 2 * nj], F16, tag="h2", bufs=5)
                  for kc in range(2):
                      psC = ps.tile([128, 2 * nj], F32, tag="ac", bufs=3)
                      for hc in range(4):
                          nc.tensor.matmul(
                              psC[:],
                              W2_s[:, hc, kc * 128 : (kc + 1) * 128],
                              h1T[:, hc, :],
                              start=(hc == 0),
                              stop=(hc == 3),
                          )
                      nc.scalar.activation(
                          h2T[:, kc, :], psC[:], AF.Relu, bias=b2_s[:, kc : kc + 1]
                      )
                  return h2T

              strip2 = [None]

              def stage_D(p, h2T):
                  psD = ps.tile([1, 2 * nj], F32, tag="d", bufs=1)
                  nc.tensor.matmul(psD[:], W3_s[:, 0:1], h2T[:, 0, :], start=True, stop=False)
                  nc.tensor.matmul(psD[:], W3_s[:, 1:2], h2T[:, 1, :], start=False, stop=True)
                  # engines cannot write at partition base p, only DMA can
                  # shift partitions; strips are copied out two pairs at a
                  # time on Pool's software DGE
                  if p % 2 == 0:
                      strip2[0] = wp.tile([1, 2, 2 * nj], F32, tag="st", name="strip2")
                  nc.scalar.activation(strip2[0][:, p % 2, :], psD[:], AF.Identity, bias=b3_s[:])
                  if p % 2 == 1:
                      nc.gpsimd.dma_start(logits_sb[p - 1 : p + 1, :], strip2[0][:])

              # epilogue split at pair 32: engine partition bases must be
              # 0/32/64/96
              cmap_sb = cp.tile([npair, 2 * nj], F32)
              half_pairs = min(32, npair)
              logits_ov = logits_o.ap().rearrange("(p i) j -> p (i j)", i=2)
              cmap_ov = cmap_o.ap().rearrange("(p i) j -> p (i j)", i=2)

              def epilogue_half(h):
                  psl = slice(0, half_pairs) if h == 0 else slice(half_pairs, npair)
                  if psl.start >= psl.stop:
                      return
                  # logits dep is ready when emitted; cmap DMA goes on the Act
                  # queue right behind its sigmoid so SP's tp triggers for the
                  # second half are never blocked waiting on it
                  nc.sync.dma_start(logits_ov[psl, :], logits_sb[psl, :])
                  nc.scalar.activation(cmap_sb[psl, :], logits_sb[psl, :], AF.Sigmoid)
                  nc.scalar.dma_start(cmap_ov[psl, :], cmap_sb[psl, :])

              tp_fetch(0)
              tp_fetch(1)
              h1q, h2q = [], []

              def drain_c():
                  q, h1 = h1q.pop(0)
                  h2q.append((q, stage_C(q, h1)))

              def drain_d():
                  q, h2 = h2q.pop(0)
                  stage_D(q, h2)
                  if q == half_pairs - 1:
                      epilogue_half(0)

              for p in range(npair):
                  h1q.append((p, stage_B(p)))
                  if len(h1q) > 2:
                      drain_c()
                  if len(h2q) > 2:
                      drain_d()
              while h1q:
                  drain_c()
              while h2q:
                  drain_d()
              epilogue_half(1)

    nc.compile()
    return nc


def _active_idx(motif_mask):
    return [np.nonzero(np.asarray(motif_mask[b]) > 0.5)[0] for b in range(B)]


def _core_rows(idx_b, half):
    n = len(idx_b)
    h = (n + 1) // 2
    return idx_b[:h] if half == 0 else idx_b[h:]


def _in_maps(z, motif_mask, W1, b1, W2, b2, W3, b3, rows=ROWS, nj=NJ):
    z = np.ascontiguousarray(np.asarray(z, dtype=np.float32))
    motif_mask = np.asarray(motif_mask, dtype=np.float32)
    W1 = np.ascontiguousarray(np.asarray(W1, dtype=np.float16)).reshape(D * D, H)
    W2 = np.ascontiguousarray(np.asarray(W2, dtype=np.float32)).reshape(H, H // 2)
    W3 = np.ascontiguousarray(np.asarray(W3, dtype=np.float32)).reshape(H // 2, 1)
    b1 = np.ascontiguousarray(np.asarray(b1, dtype=np.float32)).reshape(H)
    b2 = np.ascontiguousarray(np.asarray(b2, dtype=np.float32)).reshape(H // 2)
    b3 = np.ascontiguousarray(np.asarray(b3, dtype=np.float32)).reshape(1)
    idx = _active_idx(motif_mask)
    maps = []
    for c in range(NCORES):
        b, half = divmod(c, 2)
        act = idx[b]
        n = len(act)
        rows_c = _core_rows(act, half)
        zb = z[b]  # (N, D)
        ziT = np.zeros((D, rows), np.float16)
        ziT[:, : len(rows_c)] = zb[rows_c].T.astype(np.float16)
        zTx = np.zeros((D + 1, nj), np.float32)
        zTx[:D, :n] = zb[act].T
        zTx[D, :n] = 1.0
        maps.append(
            {
                "ziT": ziT,
                "zTx": zTx,
                "W1": W1,
                "W2": W2,
                "W3": W3,
                "b1": b1,
                "b2": b2,
                "b3": b3,
            }
        )
    return maps


def kernel(z, motif_mask, residue_mask, W1, b1, W2, b2, W3, b3):
    global _cached_nc
    motif_np = np.asarray(motif_mask, dtype=np.float32)
    idx = _active_idx(motif_np)
    max_n = max(len(ix) for ix in idx)

    if max_n <= NJ:
        rows, nj, key = ROWS, NJ, 1
    else:
        rows, nj, key = 128, 256, "full"  # universal fallback, any mask fits
    if key not in _cached_nc:
        _cached_nc[key] = _build(rows=rows, nj=nj)
    nc = _cached_nc[key]

    maps = _in_maps(z, motif_mask, W1, b1, W2, b2, W3, b3, rows=rows, nj=nj)
    res = run_bass_kernel_spmd(nc, maps, list(range(NCORES)))

    # masked entries are exact: logit = 0, cmap = sigmoid(0) = 0.5
    logits = np.zeros((B, N, N), np.float32)
    cmap = np.full((B, N, N), 0.5, np.float32)
    for c in range(NCORES):
        b, half = divmod(c, 2)
        act = idx[b]
        n = len(act)
        rows_c = _core_rows(act, half)
        if len(rows_c) == 0:
            continue
        lg = res.results[c]["logits"][: len(rows_c), :n]
        cm = res.results[c]["cmap"][: len(rows_c), :n]
        logits[b][np.ix_(rows_c, act)] = lg
        cmap[b][np.ix_(rows_c, act)] = cm
    return cmap, logits
